# revision 12
# baseline (speedup 1.0000x reference)
"""Trainium2 Bass kernel for nn_BCNLayer (locally-connected 7x7 lattice layer + sigmoid).

Math: y[i,j,b] = sigmoid( sum_{dy,dx in [-3,3]} w[dy+3,dx+3][(i-dy)*W + (j-dx)]
                          * x[(i-dy)*W + (j-dx), b] )   (zero outside lattice)

Strategy:
  - 8-way shard over lattice rows (H=128 -> 16 dest rows/core, 22 source rows
    with 3-row halos, zero-padded at the edges).
  - For one dest row i and source-row offset d (7 of them), the contribution is
    a banded 128x128 matrix (band +-3 over lattice columns) applied to the
    source row's [128 cols x B batch] slab: nc.tensor.matmul(psum, lhsT=Wband,
    rhs=xrow) accumulated over the 7 source rows.  Banded matrices are prebuilt
    on the host and DMA'd in.
  - PE is the bottleneck (224 matmuls x 512 rows at 2.4GHz ~= 48.4us/core), so
    everything is scheduled around keeping it 100% busy from the earliest start:
      * tile order is t-outer / batch-chunk-inner so each wb tile serves two
        consecutive psum tiles (halves weight-stream bandwidth demand).
      * critical-first loads: tile 0's operands split across all three DMA
        paths (SP HWDGE, ACT HWDGE, Pool SWDGE) with nothing else competing.
      * bulk loads are paced: x row-groups are interleaved into the compute
        stream (engine FIFOs release them just-in-time), wb tiles gated on
        matmul progress via explicit deps, so in-flight DMA never floods the
        HBM path that the next-needed transfer is on.
      * spin matmuls keep the PE busy through the load phase so the HAM clock
        gate is warm (2.4GHz) when real matmuls start.
      * outputs are bf16 (host upcasts; bf16 keeps fp32's exponent range so
        tiny sigmoid outputs stay accurate) on the ACT/SP HWDGE rings.
"""

import os

import numpy as np

H = 128
W = 128
HW = H * W
B = 1024
NCORES = 8
T = H // NCORES  # dest rows per core = 16
S = T + 6        # source rows per core (halo 3 each side) = 22
BC = 512         # batch chunk (psum bank = 512 fp32)
NB = B // BC     # chunks = 2
NSPIN = 95       # PE warm-up matmuls during the load phase

_cache: dict = {}

# filled by the last kernel() call when KERNEL_TRACE=1
last_exec_time_ns = None
last_results = None


def _build_program():
    from contextlib import ExitStack

    import concourse.bacc as bacc
    import concourse.mybir as mybir
    import concourse.tile as tile
    from concourse.tile_rust import add_dep_helper

    nc = bacc.Bacc(
        "TRN2", target_bir_lowering=False, debug=False, num_devices=NCORES
    )
    f16 = mybir.dt.float16
    bf16 = mybir.dt.bfloat16
    f32 = mybir.dt.float32

    xs = nc.dram_tensor("xs", [128, NB, S, BC], f16, kind="ExternalInput").ap()
    wb = nc.dram_tensor("wb", [128, T * 7 * 128], f16, kind="ExternalInput").ap()
    y = nc.dram_tensor("y", [T, NB, 128, BC], bf16, kind="ExternalOutput").ap()

    KW = 7 * 128

    with tile.TileContext(nc) as tc, ExitStack() as ctx:
        xpool = ctx.enter_context(tc.tile_pool(name="x", bufs=1))
        wpool = ctx.enter_context(tc.tile_pool(name="w", bufs=1))
        ppool = ctx.enter_context(tc.tile_pool(name="ps", bufs=6, space="PSUM"))
        spool = ctx.enter_context(tc.tile_pool(name="sp", bufs=1, space="PSUM"))
        opool = ctx.enter_context(tc.tile_pool(name="o", bufs=6))

        xt = xpool.tile([128, NB * S * BC], f16, tag="xslab")
        xt4 = xt[:].rearrange("p (c s b) -> p c s b", c=NB, s=S)
        wt = wpool.tile([128, T * KW], f16, tag="wslab")
        wsp = wpool.tile([128, 64], f16, tag="wspin")
        psp = spool.tile([128, 64], f32, tag="pspin")
        warm = opool.tile([128, 1], f32, tag="warm")

        # ---- PE warm-up spins (HAM clock gate needs ~3.4us of PE busy).
        nc.vector.memset(wsp[:], 0.0)
        nc.vector.memset(warm[:], 0.0)
        for _ in range(NSPIN):
            nc.tensor.matmul(
                psp[0:64, :], wsp[:, 0:64], wsp[:], start=True, stop=True
            )

        # Every DMA ring is an in-order queue only if we SAY so: the Tile
        # scheduler is free to reorder same-engine instructions, so chain
        # each ring explicitly (a paced DMA scheduled ahead of the critical
        # loads on its ring would otherwise head-of-line deadlock).
        last_on = {}

        def ring_dma(ring, out, in_):
            eng = {"sync": nc.sync, "scalar": nc.scalar, "pool": nc.gpsimd}[ring]
            d = eng.dma_start(out=out, in_=in_)
            if ring in last_on:
                add_dep_helper(d.ins, last_on[ring].ins, False, f"{ring} order")
            last_on[ring] = d
            return d

        # ---- preload: rows 0-7 of both chunks + c0 rows 8-9 + wt0/wt1,
        # 2-row granular, ordered by urgency per ring.  The tile sequence
        # below starts with (c0,t0),(c0,t1) so compute begins once c0 rows
        # 0-7 + wt0/wt1 land (~11.2us) while the c1 rows stream behind.
        ring_dma("sync", xt4[:, 0, 0:2, :], xs[:, 0, 0:2, :])
        ring_dma("scalar", xt4[:, 0, 2:4, :], xs[:, 0, 2:4, :])
        ring_dma("pool", wt[:, 0:KW], wb[:, 0:KW])
        ring_dma("sync", xt4[:, 0, 4:6, :], xs[:, 0, 4:6, :])
        ring_dma("scalar", xt4[:, 0, 6:8, :], xs[:, 0, 6:8, :])
        ring_dma("pool", wt[:, KW : 2 * KW], wb[:, KW : 2 * KW])
        ring_dma("sync", xt4[:, 1, 0:2, :], xs[:, 1, 0:2, :])
        ring_dma("scalar", xt4[:, 1, 2:4, :], xs[:, 1, 2:4, :])
        ring_dma("sync", xt4[:, 1, 4:6, :], xs[:, 1, 4:6, :])
        ring_dma("scalar", xt4[:, 1, 6:8, :], xs[:, 1, 6:8, :])
        ring_dma("sync", xt4[:, 0, 8:10, :], xs[:, 0, 8:10, :])
        # sigmoid table warm-up (ACT loads its table during the load phase)
        nc.scalar.activation(warm[:], warm[:], mybir.ActivationFunctionType.Sigmoid)

        # wb tiles 2..15: on the Pool SWDGE ring, gated on matmul progress
        # so they never compete with nearer-term transfers.
        wb_dmas = {}
        for t in range(2, T):
            wb_dmas[t] = ring_dma(
                "pool", wt[:, t * KW : (t + 1) * KW], wb[:, t * KW : (t + 1) * KW]
            )

        # Remaining x row-groups, interleaved into the compute stream ~2
        # pairs ahead of first use (chained SP ring releases them in order).
        xgroup_at = {
            2: [(1, 8, 11), (0, 10, 13)],
            3: [(1, 11, 14)],
            5: [(0, 13, 16)],
            6: [(1, 14, 17)],
            8: [(0, 16, 19)],
            9: [(1, 17, 20)],
            11: [(0, 19, 22)],
            12: [(1, 20, 22)],
        }

        # staggered start, then chunk-paired so each wb tile serves both
        # chunks back-to-back
        seq = [(0, 0), (0, 1), (1, 0), (1, 1)] + [
            (c, t) for t in range(2, T) for c in range(NB)
        ]
        emitted_xg = set()
        first_mm = {}
        last_ct = seq[-1]
        for c, t in seq:
            if t in xgroup_at and (c, t) not in emitted_xg and c == 0:
                emitted_xg.add((c, t))
                for cg, lo, hi in xgroup_at[t]:
                    ring_dma("sync", xt4[:, cg, lo:hi, :], xs[:, cg, lo:hi, :])
            ps = ppool.tile([128, BC], f32, tag="ps")
            for d in range(7):
                lhs = wt[:, (t * 7 + d) * 128 : (t * 7 + d + 1) * 128]
                rhs = xt4[:, c, t + d, :]
                mm = nc.tensor.matmul(
                    ps[:], lhs, rhs, start=(d == 0), stop=(d == 6)
                )
                if c == 0 and d == 0:
                    first_mm[t] = mm
            ot = opool.tile([128, BC], bf16, tag="o")
            if (c, t) == last_ct:
                # split the final sigmoid so its first half's store overlaps
                # the second half's activation (shorter drain tail)
                HB = BC // 2
                nc.scalar.activation(
                    ot[:, 0:HB], ps[:, 0:HB], mybir.ActivationFunctionType.Sigmoid
                )
                ring_dma("scalar", y[t, c, :, 0:HB], ot[:, 0:HB])
                nc.scalar.activation(
                    ot[:, HB:BC], ps[:, HB:BC], mybir.ActivationFunctionType.Sigmoid
                )
                ring_dma("sync", y[t, c, :, HB:BC], ot[:, HB:BC])
            else:
                nc.scalar.activation(
                    ot[:], ps[:], mybir.ActivationFunctionType.Sigmoid
                )
                ring_dma("scalar" if c == 0 else "sync", y[t, c], ot[:])

        # pacing edges for the weight stream
        for t in range(2, T):
            add_dep_helper(
                wb_dmas[t].ins,
                first_mm[max(t - 3, 0)].ins,
                True,
                "pace wb stream behind compute",
            )
    nc.compile()
    return nc


def _build_banded(weights: np.ndarray) -> np.ndarray:
    """G[i, d, js, jd] = weight of edge (src row i+d-3, col js) -> (dest row i, col jd).

    dy = 3 - d (dest = src + dy), dx = jd - js, weight index = w[dy+3, dx+3][src_hw].
    """
    w4 = weights.reshape(7, 7, H, W)
    G = np.zeros((H, 7, W, W), np.float32)
    i = np.arange(H)
    for d in range(7):
        r = i + d - 3
        vi = i[(r >= 0) & (r < H)]
        if len(vi) == 0:
            continue
        for dxi in range(7):
            dx = dxi - 3
            js = np.arange(max(0, -dx), W - max(0, dx))
            G[vi[:, None], d, js[None, :], js[None, :] + dx] = w4[6 - d, dxi][
                (vi + d - 3)[:, None], js[None, :]
            ]
    return G


def kernel(x: np.ndarray, weights: np.ndarray) -> np.ndarray:
    global last_exec_time_ns, last_results
    from concourse.bass_utils import run_bass_kernel_spmd

    x = np.ascontiguousarray(x, dtype=np.float32)
    weights = np.ascontiguousarray(weights, dtype=np.float32)

    if "nc" not in _cache:
        _cache["nc"] = _build_program()
    nc = _cache["nc"]

    x3 = x.reshape(H, W, B)
    xp = np.zeros((H + 6, W, B), np.float16)
    xp[3 : H + 3] = x3.astype(np.float16)
    G = _build_banded(weights).astype(np.float16)  # [H, 7, W(js), W(jd)]

    in_maps = []
    for q in range(NCORES):
        blk = xp[T * q : T * q + S]  # [S, W, B]
        # -> [W(partition), NB, S, BC], contiguous
        xh = np.ascontiguousarray(
            blk.transpose(1, 0, 2).reshape(W, S, NB, BC).transpose(0, 2, 1, 3)
        )
        gq = G[T * q : T * q + T]  # [T, 7, W(js), W(jd)]
        # -> [W(js) partition, T*7*W(jd)], contiguous
        wh = np.ascontiguousarray(
            gq.transpose(2, 0, 1, 3).reshape(W, T * 7 * W)
        )
        in_maps.append({"xs": xh, "wb": wh})

    trace = os.environ.get("KERNEL_TRACE", "0") == "1"
    res = run_bass_kernel_spmd(
        nc, in_maps, core_ids=list(range(NCORES)), trace=trace
    )
    last_exec_time_ns = res.exec_time_ns
    last_results = res
    parts = []
    for r in res.results:
        arr = np.asarray(r["y"])  # [T, NB, 128, BC] bf16
        parts.append(
            arr.transpose(0, 2, 1, 3).reshape(T * W, B).astype(np.float32)
        )
    return np.concatenate(parts, axis=0)


# revision 15
# speedup vs baseline: 1.0596x; 1.0596x over previous
"""Trainium2 Bass kernel for nn_BCNLayer (locally-connected 7x7 lattice layer + sigmoid).

Math: y[i,j,b] = sigmoid( sum_{dy,dx in [-3,3]} w[dy+3,dx+3][(i-dy)*W + (j-dx)]
                          * x[(i-dy)*W + (j-dx), b] )   (zero outside lattice)

Strategy:
  - 8-way shard over lattice rows (H=128 -> 16 dest rows/core, 22 source rows
    with 3-row halos, zero-padded at the edges).
  - For one dest row i and source-row offset d (7 of them), the contribution is
    a banded 128x128 matrix (band +-3 over lattice columns) applied to the
    source row's [128 cols x B batch] slab: nc.tensor.matmul(psum, lhsT=Wband,
    rhs=xrow) accumulated over the 7 source rows.  Banded matrices are prebuilt
    on the host and DMA'd in.
  - PE is the bottleneck (224 matmuls x 512 rows at 2.4GHz ~= 48.4us/core), so
    everything is scheduled around keeping it 100% busy from the earliest start:
      * tile order is t-outer / batch-chunk-inner so each wb tile serves two
        consecutive psum tiles (halves weight-stream bandwidth demand).
      * critical-first loads: tile 0's operands split across all three DMA
        paths (SP HWDGE, ACT HWDGE, Pool SWDGE) with nothing else competing.
      * bulk loads are paced: x row-groups are interleaved into the compute
        stream (engine FIFOs release them just-in-time), wb tiles gated on
        matmul progress via explicit deps, so in-flight DMA never floods the
        HBM path that the next-needed transfer is on.
      * spin matmuls keep the PE busy through the load phase so the HAM clock
        gate is warm (2.4GHz) when real matmuls start.
      * outputs are bf16 (host upcasts; bf16 keeps fp32's exponent range so
        tiny sigmoid outputs stay accurate) on the ACT/SP HWDGE rings.
"""

import os

import numpy as np

H = 128
W = 128
HW = H * W
B = 1024
NCORES = 8
T = H // NCORES  # dest rows per core = 16
S = T + 6        # source rows per core (halo 3 each side) = 22
BC = 512         # batch chunk (psum bank = 512 fp32)
NB = B // BC     # chunks = 2
NSPIN = 95       # PE warm-up matmuls during the load phase

_cache: dict = {}

# filled by the last kernel() call when KERNEL_TRACE=1
last_exec_time_ns = None
last_results = None


def _build_program():
    from contextlib import ExitStack

    import concourse.bacc as bacc
    import concourse.mybir as mybir
    import concourse.tile as tile
    from concourse.tile_rust import add_dep_helper

    nc = bacc.Bacc(
        "TRN2", target_bir_lowering=False, debug=False, num_devices=NCORES
    )
    f16 = mybir.dt.float16
    bf16 = mybir.dt.bfloat16
    f32 = mybir.dt.float32

    xs = nc.dram_tensor("xs", [128, NB, S, BC], f16, kind="ExternalInput").ap()
    wb = nc.dram_tensor("wb", [128, T * 7 * 128], f16, kind="ExternalInput").ap()
    y = nc.dram_tensor("y", [T, NB, 128, BC], bf16, kind="ExternalOutput").ap()

    KW = 7 * 128

    with tile.TileContext(nc) as tc, ExitStack() as ctx:
        xpool = ctx.enter_context(tc.tile_pool(name="x", bufs=1))
        wpool = ctx.enter_context(tc.tile_pool(name="w", bufs=1))
        ppool = ctx.enter_context(tc.tile_pool(name="ps", bufs=6, space="PSUM"))
        spool = ctx.enter_context(tc.tile_pool(name="sp", bufs=1, space="PSUM"))
        opool = ctx.enter_context(tc.tile_pool(name="o", bufs=6))

        xt = xpool.tile([128, NB * S * BC], f16, tag="xslab")
        xt4 = xt[:].rearrange("p (c s b) -> p c s b", c=NB, s=S)
        wt = wpool.tile([128, T * KW], f16, tag="wslab")
        wsp = wpool.tile([128, 64], f16, tag="wspin")
        psp = spool.tile([128, 64], f32, tag="pspin")
        warm = opool.tile([128, 1], f32, tag="warm")

        # ---- PE warm-up spins (HAM clock gate needs ~3.4us of PE busy).
        nc.vector.memset(wsp[:], 0.0)
        nc.vector.memset(warm[:], 0.0)
        for _ in range(NSPIN):
            nc.tensor.matmul(
                psp[0:64, :], wsp[:, 0:64], wsp[:], start=True, stop=True
            )

        # Every DMA ring is an in-order queue only if we SAY so: the Tile
        # scheduler is free to reorder same-engine instructions, so chain
        # each ring explicitly (a paced DMA scheduled ahead of the critical
        # loads on its ring would otherwise head-of-line deadlock).
        last_on = {}

        def ring_dma(ring, out, in_):
            eng = {"sync": nc.sync, "scalar": nc.scalar, "pool": nc.gpsimd}[ring]
            d = eng.dma_start(out=out, in_=in_)
            if ring in last_on:
                add_dep_helper(d.ins, last_on[ring].ins, False, f"{ring} order")
            last_on[ring] = d
            return d

        # ---- preload: rows 0-7 of both chunks + c0 rows 8-9 + wt0/wt1,
        # 2-row granular, ordered by urgency per ring.  The tile sequence
        # below starts with (c0,t0),(c0,t1) so compute begins once c0 rows
        # 0-7 + wt0/wt1 land (~11.2us) while the c1 rows stream behind.
        ring_dma("sync", xt4[:, 0, 0:2, :], xs[:, 0, 0:2, :])
        ring_dma("scalar", xt4[:, 0, 2:4, :], xs[:, 0, 2:4, :])
        ring_dma("pool", wt[:, 0:KW], wb[:, 0:KW])
        ring_dma("sync", xt4[:, 0, 4:6, :], xs[:, 0, 4:6, :])
        ring_dma("scalar", xt4[:, 0, 6:8, :], xs[:, 0, 6:8, :])
        ring_dma("pool", wt[:, KW : 2 * KW], wb[:, KW : 2 * KW])
        ring_dma("sync", xt4[:, 1, 0:2, :], xs[:, 1, 0:2, :])
        ring_dma("scalar", xt4[:, 1, 2:4, :], xs[:, 1, 2:4, :])
        ring_dma("sync", xt4[:, 1, 4:6, :], xs[:, 1, 4:6, :])
        ring_dma("scalar", xt4[:, 1, 6:8, :], xs[:, 1, 6:8, :])
        ring_dma("sync", xt4[:, 0, 8:10, :], xs[:, 0, 8:10, :])
        # sigmoid table warm-up (ACT loads its table during the load phase)
        nc.scalar.activation(warm[:], warm[:], mybir.ActivationFunctionType.Sigmoid)
        # next two row-groups ride the ACT ring's early slack (it is idle
        # between its preload and the first sigmoid at ~12.6us)
        ring_dma("scalar", xt4[:, 1, 8:11, :], xs[:, 1, 8:11, :])
        ring_dma("scalar", xt4[:, 0, 10:13, :], xs[:, 0, 10:13, :])

        # wb tiles 2..15: on the Pool SWDGE ring, gated on matmul progress
        # so they never compete with nearer-term transfers.
        wb_dmas = {}
        for t in range(2, T):
            wb_dmas[t] = ring_dma(
                "pool", wt[:, t * KW : (t + 1) * KW], wb[:, t * KW : (t + 1) * KW]
            )

        # Remaining x row-groups, interleaved into the compute stream ~2
        # pairs ahead of first use, alternating rings (chained ring order
        # releases them just-in-time).
        xgroup_at = {
            3: [("sync", 1, 11, 14)],
            5: [("scalar", 0, 13, 16)],
            6: [("sync", 1, 14, 17)],
            8: [("scalar", 0, 16, 19)],
            9: [("sync", 1, 17, 20)],
            11: [("scalar", 0, 19, 22)],
            12: [("sync", 1, 20, 22)],
        }

        # staggered start, then chunk-paired so each wb tile serves both
        # chunks back-to-back
        seq = [(0, 0), (0, 1), (1, 0), (1, 1)] + [
            (c, t) for t in range(2, T) for c in range(NB)
        ]
        first_mm = {}
        for c, t in seq:
            if t in xgroup_at and c == 0:
                for ring, cg, lo, hi in xgroup_at[t]:
                    ring_dma(ring, xt4[:, cg, lo:hi, :], xs[:, cg, lo:hi, :])
            ps = ppool.tile([128, BC], f32, tag="ps")
            for d in range(7):
                lhs = wt[:, (t * 7 + d) * 128 : (t * 7 + d + 1) * 128]
                rhs = xt4[:, c, t + d, :]
                mm = nc.tensor.matmul(
                    ps[:], lhs, rhs, start=(d == 0), stop=(d == 6)
                )
                if c == 0 and d == 0:
                    first_mm[t] = mm
            ot = opool.tile([128, BC], bf16, tag="o")
            nc.scalar.activation(
                ot[:], ps[:], mybir.ActivationFunctionType.Sigmoid
            )
            ring_dma("scalar" if c == 0 else "sync", y[t, c], ot[:])

        # pacing edges for the weight stream
        for t in range(2, T):
            add_dep_helper(
                wb_dmas[t].ins,
                first_mm[max(t - 3, 0)].ins,
                True,
                "pace wb stream behind compute",
            )
    nc.compile()
    return nc


def _build_banded(weights: np.ndarray) -> np.ndarray:
    """G[i, d, js, jd] = weight of edge (src row i+d-3, col js) -> (dest row i, col jd).

    dy = 3 - d (dest = src + dy), dx = jd - js, weight index = w[dy+3, dx+3][src_hw].
    """
    w4 = weights.reshape(7, 7, H, W)
    G = np.zeros((H, 7, W, W), np.float32)
    i = np.arange(H)
    for d in range(7):
        r = i + d - 3
        vi = i[(r >= 0) & (r < H)]
        if len(vi) == 0:
            continue
        for dxi in range(7):
            dx = dxi - 3
            js = np.arange(max(0, -dx), W - max(0, dx))
            G[vi[:, None], d, js[None, :], js[None, :] + dx] = w4[6 - d, dxi][
                (vi + d - 3)[:, None], js[None, :]
            ]
    return G


def kernel(x: np.ndarray, weights: np.ndarray) -> np.ndarray:
    global last_exec_time_ns, last_results
    from concourse.bass_utils import run_bass_kernel_spmd

    x = np.ascontiguousarray(x, dtype=np.float32)
    weights = np.ascontiguousarray(weights, dtype=np.float32)

    if "nc" not in _cache:
        _cache["nc"] = _build_program()
    nc = _cache["nc"]

    x3 = x.reshape(H, W, B)
    xp = np.zeros((H + 6, W, B), np.float16)
    xp[3 : H + 3] = x3.astype(np.float16)
    G = _build_banded(weights).astype(np.float16)  # [H, 7, W(js), W(jd)]

    in_maps = []
    for q in range(NCORES):
        blk = xp[T * q : T * q + S]  # [S, W, B]
        # -> [W(partition), NB, S, BC], contiguous
        xh = np.ascontiguousarray(
            blk.transpose(1, 0, 2).reshape(W, S, NB, BC).transpose(0, 2, 1, 3)
        )
        gq = G[T * q : T * q + T]  # [T, 7, W(js), W(jd)]
        # -> [W(js) partition, T*7*W(jd)], contiguous
        wh = np.ascontiguousarray(
            gq.transpose(2, 0, 1, 3).reshape(W, T * 7 * W)
        )
        in_maps.append({"xs": xh, "wb": wh})

    trace = os.environ.get("KERNEL_TRACE", "0") == "1"
    res = run_bass_kernel_spmd(
        nc, in_maps, core_ids=list(range(NCORES)), trace=trace
    )
    last_exec_time_ns = res.exec_time_ns
    last_results = res
    parts = []
    for r in res.results:
        arr = np.asarray(r["y"])  # [T, NB, 128, BC] bf16
        parts.append(
            arr.transpose(0, 2, 1, 3).reshape(T * W, B).astype(np.float32)
        )
    return np.concatenate(parts, axis=0)
